# revision 55
# baseline (speedup 1.0000x reference)
"""DeepseekV3 MLA attention (B=2, S=2048, D=2048, H=16) on 8 trn2 NeuronCores.

Sharding: data-parallel over batch x tensor-parallel over heads.
Core c handles batch b=c//4 and heads [4*(c%4) .. 4*(c%4)+4).

v2 pipeline (vs baseline): collectives are split and overlapped with compute.
  stage A (token-sharded): kv A-proj first -> AG of RAW kv activations
  (+k_pe rows +inv_kv row) issued early, overlapping the q A-proj; then AG of
  RAW q activations (+inv_q row).  RMSNorm inv factors travel with the AG and
  are applied AFTER the B-projections (per-token column scaling commutes with
  the rank contraction), so stage A never serializes on the norm.
  stage B: kv-side work (kTn, V, k-rope) runs under the q AllGather; q_b
  follows when the q AG lands.
  attention: transposed per k-tile as baseline, but the softmax-normalize
  chain is ones-broadcast-matmul -> reciprocal_approx_fast on the [128,512]
  broadcast -> one fused multiply (no serial [1,512] reciprocal), with a
  dedicated PSUM bank for the broadcast so consecutive (head, q-block) units
  pipeline without stalling the PE.
  After each head finishes, its attnT is AllGathered immediately (4 chunked
  collectives hidden under the remaining heads' attention compute).
  o-proj: full contract over 16 gathered heads for this core's own 512-token
  slice, as baseline.

Host side only shards/transposes/concats (weight folding of the RMSNorm
gains and the softmax scale is compile-time weight prep).
"""

import numpy as np

import concourse.bass as bass
import concourse.mybir as mybir
import concourse.tile as tile
from concourse.bass_utils import run_bass_kernel_spmd

F32 = mybir.dt.float32
F16 = mybir.dt.float16
AF = mybir.ActivationFunctionType

B, S, D = 2, 2048, 2048
H = 16
NOPE, ROPE, VDIM = 128, 64, 128
QHD = NOPE + ROPE
QR, KVR = 1536, 512
THETA = 10000.0
EPS = 1e-6
SCALE = QHD ** -0.5

HPG = 4          # heads per group (per core)
NST = 4          # 512-token stiles
ST = 512
NDC = D // 128   # 16 d-chunks
NRC = QR // 128  # 12 rank chunks (q)
NKC = KVR // 128 # 4 rank chunks (kv)
NTT = S // 128   # 16 token tiles
GROUPS = [[0, 1, 2, 3], [4, 5, 6, 7]]
AGKV_R = KVR + ROPE + 1   # raw ckv rows + k_pe rows + inv_kv row
AGQ_RS = (QR // 3, QR // 3, QR // 3 + 1)  # raw q rows per chunk; inv rides last
NQP = 3                   # q chunks
QPC = NRC // NQP          # rank chunks per q chunk


def _split_multi_waits(nc):
    """walrus in this container accepts only ONE sem wait per instruction;
    split extras onto same-engine NOPs placed immediately before."""
    ctr = 0
    for bb in nc.main_func.blocks:
        new = []
        for ins in bb.instructions:
            si = ins.sync_info
            if si is not None and len(si.on_wait) > 1:
                waits = list(si.on_wait)
                for w in waits[:-1]:
                    nop = mybir.InstNoOp(name=f"I-ws{ctr}", ins=[], outs=[])
                    ctr += 1
                    nop.engine = ins.engine
                    nop.sync_info = mybir.SyncInfo(on_wait=[w], on_update=[])
                    new.append(nop)
                si.on_wait = [waits[-1]]
                ins.sync_info = si
            new.append(ins)
        bb.instructions = new


def _build_program(mask_mode):
    """mask_mode: 'causal' | 'none' | 'generic'"""
    nc = bass.Bass()

    hT_d = nc.dram_tensor("hiddenT", [D, ST], F16, kind="ExternalInput")
    wqa_d = nc.dram_tensor("wqa", [D, QR], F16, kind="ExternalInput")
    wkva_d = nc.dram_tensor("wkva", [D, KVR + ROPE], F16, kind="ExternalInput")
    wqbn_d = nc.dram_tensor("wqbn", [QR, HPG * NOPE], F16, kind="ExternalInput")
    wqbr_d = nc.dram_tensor("wqbr", [QR, HPG * ROPE], F16, kind="ExternalInput")
    wkvbk_d = nc.dram_tensor("wkvbk", [KVR, HPG * NOPE], F16, kind="ExternalInput")
    wkvbv_d = nc.dram_tensor("wkvbv", [KVR, HPG * VDIM], F16, kind="ExternalInput")
    wo_d = nc.dram_tensor("wo", [H * VDIM, D], F16, kind="ExternalInput")
    cos2_d = nc.dram_tensor("cos2", [2 * ROPE, S], F16, kind="ExternalInput")
    sin2_d = nc.dram_tensor("sin2", [2 * ROPE, S], F16, kind="ExternalInput")
    if mask_mode == "causal":
        pmask_d = nc.dram_tensor("pmaskT", [4, 128, ST], F16, kind="ExternalInput")
    if mask_mode == "generic":
        maskT_d = nc.dram_tensor("maskT", [S, S], F32, kind="ExternalInput")
    o_d = nc.dram_tensor("o_part", [ST, D], F16, kind="ExternalOutput")

    with tile.TileContext(nc) as tc:
        with (
            tc.tile_pool(name="const", bufs=1) as pco,
            tc.tile_pool(name="persist", bufs=1) as pp,
            tc.tile_pool(name="dram", bufs=1, space="DRAM") as pdr,
        ):
            ones_col = pco.tile([128, 1], F16)
            nc.vector.memset(ones_col[:], 1.0)
            ones_row = pco.tile([1, 128], F16)
            nc.vector.memset(ones_row[:], 1.0)
            epst = pco.tile([1, 1], F32)
            nc.vector.memset(epst[:], EPS)
            # warm the Exp activation table before attention needs it
            warme = pco.tile([1, 1], F16)
            nc.scalar.activation(warme[:], epst[:], AF.Exp)

            # persistent activation tensors
            qTn = [pp.tile([128, S], F16, name=f"qTn{i}", tag=f"qTn{i}") for i in range(HPG)]
            qTr_raw = [pp.tile([128, S], F16, name=f"qTrr{i}", tag=f"qTrr{i}") for i in range(2)]
            kTn = [pp.tile([128, S], F16, name=f"kTn{i}", tag=f"kTn{i}") for i in range(HPG)]
            Vn = [pp.tile([128, HPG * VDIM], F16, name=f"V{i}", tag=f"V{i}") for i in range(NTT)]
            kpe_raw = pp.tile([ROPE, S], F16)

            # DRAM bounce buffers for the activation AllGathers
            agkv_src = pdr.tile([AGKV_R, ST], F16, name="agkv_src", tag="agkv_src")
            agkv_dst = pdr.tile([NST, AGKV_R, ST], F16, name="agkv_dst", tag="agkv_dst")
            agq_src = [pdr.tile([r, ST], F16, name=f"agq_src{c}", tag=f"agq_src{c}")
                       for c, r in enumerate(AGQ_RS)]
            agq_dst = [pdr.tile([NST, r, ST], F16, name=f"agq_dst{c}", tag=f"agq_dst{c}")
                       for c, r in enumerate(AGQ_RS)]

            # B-projection weights: pool opened early so their DMAs stream on
            # the sync queue during stage A compute
            wB_pool = tc.tile_pool(name="wB", bufs=1)
            pwb = wB_pool.__enter__()

            # ---------------- stage A: own stile only, RAW + inv rows ----------------
            with (
                tc.tile_pool(name="wA", bufs=1) as pw,
                tc.tile_pool(name="loopA", bufs=2) as pl,
                tc.tile_pool(name="loopA1", bufs=2) as pl1,
                tc.tile_pool(name="rawA", bufs=4) as pr,
                tc.tile_pool(name="psA", bufs=1, space="PSUM") as psm,
                tc.tile_pool(name="psRow", bufs=1, space="PSUM") as psr,
            ):
                # hidden stile first (critical path; SWDGE casts f32->f16
                # during the DMA), then kv-A weights (kv path runs first),
                # then q-A weights
                ht = []
                for dc in range(NDC):
                    h16 = pw.tile([128, ST], F16, name=f"ht{dc}", tag=f"ht{dc}")
                    eng = nc.gpsimd if dc % 2 == 0 else nc.scalar
                    eng.dma_start(h16[:], hT_d[dc * 128:(dc + 1) * 128, :])
                    ht.append(h16)
                wkva = [pw.tile([128, KVR + ROPE], F16, name=f"wkva{dc}", tag=f"wkva{dc}") for dc in range(NDC)]
                for dc in range(NDC):
                    nc.sync.dma_start(wkva[dc][:], wkva_d[dc * 128:(dc + 1) * 128, :])
                wqa = [pw.tile([128, QR], F16, name=f"wqa{dc}", tag=f"wqa{dc}") for dc in range(NDC)]
                for dc in range(NDC):
                    nc.sync.dma_start(wqa[dc][:], wqa_d[dc * 128:(dc + 1) * 128, :])
                # stage-B weights stream in behind the A-weights
                wkvbk = [pwb.tile([128, HPG * NOPE], F16, name=f"wkvbk{rc}", tag=f"wkvbk{rc}") for rc in range(NKC)]
                wkvbv = [pwb.tile([128, HPG * VDIM], F16, name=f"wkvbv{rc}", tag=f"wkvbv{rc}") for rc in range(NKC)]
                for rc in range(NKC):
                    nc.sync.dma_start(wkvbk[rc][:], wkvbk_d[rc * 128:(rc + 1) * 128, :])
                    nc.sync.dma_start(wkvbv[rc][:], wkvbv_d[rc * 128:(rc + 1) * 128, :])
                wqbn = [pwb.tile([128, HPG * NOPE], F16, name=f"wqbn{rc}", tag=f"wqbn{rc}") for rc in range(NRC)]
                wqbr = [pwb.tile([128, HPG * ROPE], F16, name=f"wqbr{rc}", tag=f"wqbr{rc}") for rc in range(NRC)]
                for rc in range(NRC):
                    nc.sync.dma_start(wqbn[rc][:], wqbn_d[rc * 128:(rc + 1) * 128, :])
                    nc.sync.dma_start(wqbr[rc][:], wqbr_d[rc * 128:(rc + 1) * 128, :])

                # ---- A-proj ckv + k_pe raw, dc-outer so the PE starts on the
                # first hidden chunk (5 live psums)
                pskv = [psm.tile([128, ST], F32, name=f"psA{j}", tag=f"psA{j}", bufs=1)
                        for j in range(NKC)]
                psp = psm.tile([ROPE, ST], F32, name="psRope", tag="psRope", bufs=1)
                for dc in range(NDC):
                    for rc in range(NKC):
                        nc.tensor.matmul(
                            pskv[rc][:], wkva[dc][:, rc * 128:(rc + 1) * 128], ht[dc][:],
                            start=(dc == 0), stop=(dc == NDC - 1))
                    nc.tensor.matmul(psp[:], wkva[dc][:, KVR:KVR + ROPE], ht[dc][:],
                                     start=(dc == 0), stop=(dc == NDC - 1))
                pss_kv = psr.tile([1, ST], F32, name="pss", tag="pss")
                for rc in range(NKC):
                    raw = pr.tile([128, ST], F16, name="kraw", tag="raw")
                    nc.vector.tensor_copy(raw[:], pskv[rc][:])
                    nc.scalar.dma_start(agkv_src[rc * 128:(rc + 1) * 128, :], raw[:])
                    sq = pl.tile([128, ST], F16, name="sq", tag="sq")
                    nc.vector.tensor_mul(sq[:], raw[:], raw[:])
                    nc.tensor.matmul(pss_kv[:], ones_col[:], sq[:],
                                     start=(rc == 0), stop=(rc == NKC - 1))
                kpe_s = pl1.tile([ROPE, ST], F16, name="kpe_s", tag="kpe_s")
                nc.any.tensor_copy(kpe_s[:], psp[:])
                nc.scalar.dma_start(agkv_src[KVR:KVR + ROPE, :], kpe_s[:])
                sqv = pl1.tile([1, ST], F32, name="sqv", tag="sqv")
                nc.scalar.activation(sqv[:], pss_kv[:], AF.Sqrt, scale=1.0 / KVR, bias=epst[:])
                inv = pl1.tile([1, ST], F32, name="inv", tag="inv")
                nc.vector.reciprocal_approx_fast(inv[:], sqv[:])
                inv16 = pl1.tile([1, ST], F16, name="inv16", tag="inv16")
                nc.any.tensor_copy(inv16[:], inv[:])
                nc.scalar.dma_start(agkv_src[KVR + ROPE:AGKV_R, :], inv16[:])

                # ---- AllGather raw kv activations (early, under q A-proj) ----
                nc.gpsimd.collective_compute(
                    "AllGather", mybir.AluOpType.bypass, replica_groups=GROUPS,
                    ins=[agkv_src.opt()], outs=[agkv_dst.opt()])

                # ---- A-proj q (raw): three passes of 4 rank chunks,
                # dc-outer; each pass feeds its own chunked AllGather
                pss_q = psr.tile([1, ST], F32, name="pss", tag="pss")
                for part in range(NQP):
                    psq = [psm.tile([128, ST], F32, name=f"psA{j}", tag=f"psA{j}", bufs=1)
                           for j in range(QPC)]
                    for dc in range(NDC):
                        for j in range(QPC):
                            rc = part * QPC + j
                            nc.tensor.matmul(
                                psq[j][:], wqa[dc][:, rc * 128:(rc + 1) * 128], ht[dc][:],
                                start=(dc == 0), stop=(dc == NDC - 1))
                    for j in range(QPC):
                        rc = part * QPC + j
                        raw = pr.tile([128, ST], F16, name="qraw", tag="raw")
                        nc.vector.tensor_copy(raw[:], psq[j][:])
                        nc.scalar.dma_start(agq_src[part][j * 128:(j + 1) * 128, :], raw[:])
                        sq = pl.tile([128, ST], F16, name="sq", tag="sq")
                        nc.vector.tensor_mul(sq[:], raw[:], raw[:])
                        nc.tensor.matmul(pss_q[:], ones_col[:], sq[:],
                                         start=(rc == 0), stop=(rc == NRC - 1))
                    if part < NQP - 1:
                        nc.gpsimd.collective_compute(
                            "AllGather", mybir.AluOpType.bypass, replica_groups=GROUPS,
                            ins=[agq_src[part].opt()], outs=[agq_dst[part].opt()])
                # inv_q row rides the last chunk
                sqv2 = pl1.tile([1, ST], F32, name="sqv", tag="sqv")
                nc.scalar.activation(sqv2[:], pss_q[:], AF.Sqrt, scale=1.0 / QR, bias=epst[:])
                inv2 = pl1.tile([1, ST], F32, name="inv", tag="inv")
                nc.vector.reciprocal_approx_fast(inv2[:], sqv2[:])
                inv162 = pl1.tile([1, ST], F16, name="inv16", tag="inv16")
                nc.any.tensor_copy(inv162[:], inv2[:])
                nc.scalar.dma_start(agq_src[2][AGQ_RS[2] - 1:AGQ_RS[2], :], inv162[:])

            # ---------------- stage B on gathered activations ----------------
            with (
                tc.tile_pool(name="gath", bufs=1) as pg,
                tc.tile_pool(name="bc", bufs=1) as pbc,
                tc.tile_pool(name="psB", bufs=1, space="PSUM") as psmb,
                tc.tile_pool(name="psBc", bufs=2, space="PSUM") as psbc,
            ):
                cos2 = pbc.tile([2 * ROPE, S], F16, name="cos2", tag="cos2")
                sin2 = pbc.tile([2 * ROPE, S], F16, name="sin2", tag="sin2")
                nc.sync.dma_start(cos2[:], cos2_d[:])
                nc.sync.dma_start(sin2[:], sin2_d[:])

                # gathered kv (raw) + inv rows on gpsimd: the sync queue's DMAs
                # are starved while a collective is executing, gpsimd's are not
                kg = [[pg.tile([128, ST], F16, name=f"kg{s}_{rc}", tag=f"kg{s}_{rc}")
                       for rc in range(NKC)] for s in range(NST)]
                invkv16 = [pg.tile([1, ST], F16, name=f"invkv16_{s}", tag=f"invkv16_{s}")
                           for s in range(NST)]
                # inv rows first: the broadcast/normalize chain is the longest
                # dependency ahead of the kTn matmuls (separate tiles per stile
                # so each chain only waits on its own DMA)
                for s in range(NST):
                    nc.gpsimd.dma_start(invkv16[s][:],
                                        agkv_dst[s, KVR + ROPE:AGKV_R, :])
                for s in range(NST):
                    for rc in range(NKC):
                        nc.gpsimd.dma_start(kg[s][rc][:], agkv_dst[s, rc * 128:(rc + 1) * 128, :])
                for s in range(NST):
                    nc.gpsimd.dma_start(kpe_raw[:, s * ST:(s + 1) * ST],
                                        agkv_dst[s, KVR:KVR + ROPE, :])
                # second q AllGather trigger: placed after the kv loads on the
                # gpsimd queue (triggers only wait on their staging DMAs, so
                # the collective stream order kv,q0,q1 is preserved while the
                # kv loads run on SWDGE during AG-q0)
                nc.gpsimd.collective_compute(
                    "AllGather", mybir.AluOpType.bypass, replica_groups=GROUPS,
                    ins=[agq_src[2].opt()], outs=[agq_dst[2].opt()])

                # broadcast inv_kv to 128 partitions, normalize kg in place
                bckv = [pbc.tile([128, ST], F16, name=f"bckv{s}", tag=f"bckv{s}")
                        for s in range(NST)]
                for s in range(NST):
                    psb = psbc.tile([128, ST], F32, name="psbc", tag="psbc")
                    nc.tensor.matmul(psb[:], ones_row[:], invkv16[s][:], start=True, stop=True)
                    nc.scalar.copy(bckv[s][:], psb[:])
                    for rc in range(NKC):
                        nc.vector.tensor_mul(kg[s][rc][:], kg[s][rc][:], bckv[s][:])

                # ---- k rope (vector; gpsimd is pathologically slow on
                # elementwise ops and stalls the DVE while running) ----
                kpe_both = pp.tile([128, S], F16)
                with tc.tile_pool(name="ropek", bufs=1) as prk:
                    HR = ROPE // 2  # 32
                    rot = prk.tile([ROPE, S], F16, name="rotk", tag="rotk")
                    nc.vector.tensor_scalar_mul(rot[0:HR, :], kpe_raw[HR:ROPE, :], -1.0)
                    nc.vector.tensor_copy(rot[HR:ROPE, :], kpe_raw[0:HR, :])
                    nc.vector.tensor_mul(kpe_both[0:ROPE, :], kpe_raw[:], cos2[0:ROPE, :])
                    nc.vector.tensor_mul(rot[:], rot[:], sin2[0:ROPE, :])
                    # kpe duplicated into both partition halves so the rope
                    # matmul's lhsT base_partition can match either q-rope
                    # slice (0 or 64)
                    nc.vector.tensor_add(kpe_both[0:ROPE, :], kpe_both[0:ROPE, :], rot[:])
                    nc.vector.tensor_copy(kpe_both[ROPE:2 * ROPE, :], kpe_both[0:ROPE, :])

                # ---- kTn: 4 heads x all tokens ----
                for mc in range(HPG):
                    pss4 = [psmb.tile([128, ST], F32, name=f"psB{s}", tag=f"psB{s}", bufs=1)
                            for s in range(NST)]
                    for rc in range(NKC):
                        for s in range(NST):
                            nc.tensor.matmul(
                                pss4[s][:], wkvbk[rc][:, mc * 128:(mc + 1) * 128], kg[s][rc][:],
                                start=(rc == 0), stop=(rc == NKC - 1))
                    for s in range(NST):
                        nc.vector.tensor_copy(kTn[mc][:, s * ST:(s + 1) * ST], pss4[s][:])
                # ---- V: natural layout, all token tiles ----
                for s in range(NST):
                    for tt in range(4):
                        ps = psmb.tile([128, HPG * VDIM], F32, name=f"psB{tt}", tag=f"psB{tt}", bufs=1)
                        for rc in range(NKC):
                            nc.tensor.matmul(
                                ps[:], kg[s][rc][:, tt * 128:(tt + 1) * 128], wkvbv[rc][:],
                                start=(rc == 0), stop=(rc == NKC - 1))
                        nc.vector.tensor_copy(Vn[s * 4 + tt][:], ps[:])

                # gathered q (raw), both chunks; loads issued up front so they
                # stream behind each chunk's AllGather. stiles 0-1 on scalar,
                # 2-3 on sync.
                qg = {}
                invq16 = [pg.tile([1, ST], F16, name=f"invq16_{s}", tag=f"invq16_{s}")
                          for s in range(NST)]
                for part in range(NQP):
                    if part == NQP - 1:
                        # inv rows first so the bcq chain starts immediately
                        for s in range(NST):
                            nc.gpsimd.dma_start(invq16[s][:],
                                                agq_dst[2][s, AGQ_RS[2] - 1:AGQ_RS[2], :])
                    for s in range(NST):
                        for j in range(QPC):
                            t = pg.tile([128, ST], F16, name=f"qg{part}_{s}_{j}", tag=f"qg{s}_{j}")
                            nc.gpsimd.dma_start(t[:], agq_dst[part][s, j * 128:(j + 1) * 128, :])
                            qg[(part, s, j)] = t
                # ---- q_b: chunk-outer so compute starts on the first q
                # AllGather; raw f16 partials for chunk 0, combined + norm on
                # chunk 1 ----
                qacc = [[pg.tile([128, ST], F16, name=f"qacc{s}_{m}", tag=f"qacc{s}_{m}")
                         for m in range(6)] for s in range(NST)]
                bcq = [pbc.tile([128, ST], F16, name=f"bcq{s}", tag=f"bcq{s}")
                       for s in range(NST)]
                prq = pg  # rope temporaries from the gather pool (rotating tag)
                for part in range(NQP):
                    for s in range(NST):
                        sl = slice(s * ST, (s + 1) * ST)
                        ps6 = [psmb.tile([128, ST], F32, name=f"psB{j}", tag=f"psB{j}", bufs=1)
                               for j in range(6)]
                        for j in range(QPC):
                            rc = part * QPC + j
                            for mc in range(HPG):
                                nc.tensor.matmul(
                                    ps6[mc][:], wqbn[rc][:, mc * 128:(mc + 1) * 128],
                                    qg[(part, s, j)][:],
                                    start=(j == 0), stop=(j == QPC - 1))
                            for mc in range(2):
                                nc.tensor.matmul(
                                    ps6[4 + mc][:], wqbr[rc][:, mc * 128:(mc + 1) * 128],
                                    qg[(part, s, j)][:],
                                    start=(j == 0), stop=(j == QPC - 1))
                        if part == 0:
                            for m in range(6):
                                nc.vector.tensor_copy(qacc[s][m][:], ps6[m][:])
                        elif part == 1:
                            for m in range(6):
                                nc.vector.tensor_add(qacc[s][m][:], qacc[s][m][:], ps6[m][:])
                        else:
                            for m in range(6):
                                nc.vector.tensor_add(qacc[s][m][:], qacc[s][m][:], ps6[m][:])
                            for mc in range(HPG):
                                nc.vector.tensor_mul(qTn[mc][:, sl], qacc[s][mc][:], bcq[s][:])
                            for mc in range(2):
                                nc.vector.tensor_mul(qTr_raw[mc][:, sl], qacc[s][4 + mc][:], bcq[s][:])
                            # q rope for this stile, hidden under the next
                            # stile's matmuls
                            HR = ROPE // 2
                            for i in range(2):
                                rq = prq.tile([128, ST], F16, name="rotq", tag="rotq")
                                for hh in range(2):
                                    o = hh * ROPE
                                    nc.vector.tensor_scalar_mul(
                                        rq[o:o + HR, :], qTr_raw[i][o + HR:o + ROPE, sl], -1.0)
                                    nc.vector.tensor_copy(
                                        rq[o + HR:o + ROPE, :], qTr_raw[i][o:o + HR, sl])
                                nc.vector.tensor_mul(qTr_raw[i][:, sl], qTr_raw[i][:, sl], cos2[:, sl])
                                nc.vector.tensor_mul(rq[:], rq[:], sin2[:, sl])
                                nc.vector.tensor_add(qTr_raw[i][:, sl], qTr_raw[i][:, sl], rq[:])
                    if part == NQP - 2:
                        # inv_q broadcast: placed here so the PE only waits on
                        # the last chunk's AllGather after finishing prior work
                        for s in range(NST):
                            psb = psbc.tile([128, ST], F32, name="psbc", tag="psbc")
                            nc.tensor.matmul(psb[:], ones_row[:], invq16[s][:],
                                             start=True, stop=True)
                            nc.scalar.copy(bcq[s][:], psb[:])

                qTr = qTr_raw
            wB_pool.__exit__(None, None, None)

            # ---------------- attention (transposed) + per-head AllGather ----------------
            # o-proj weights stream in under the attention compute
            oproj_pool = tc.tile_pool(name="oproj", bufs=1)
            po = oproj_pool.__enter__()
            wo = [po.tile([128, D], F16, name=f"wo{hc}", tag=f"wo{hc}") for hc in range(H)]

            attnT = [pp.tile([128, S], F16, name=f"attnT{i}", tag=f"attnT{i}") for i in range(HPG)]
            pid = nc.partition_id()
            toff = nc.snap((pid % NST) * ST, donate=True)
            atg = [po.tile([128, ST], F16, name=f"atg{hc}", tag=f"atg{hc}") for hc in range(H)]
            agat_src = [pdr.tile([VDIM, S], F16, name=f"agat_src{h}", tag=f"agat_src{h}")
                        for h in range(HPG)]
            agat_dst = [pdr.tile([NST, VDIM, S], F16, name=f"agat_dst{h}", tag=f"agat_dst{h}")
                        for h in range(HPG)]
            with (
                tc.tile_pool(name="attn", bufs=1) as pat,
                tc.tile_pool(name="ptp", bufs=8) as ptp,
                tc.tile_pool(name="bcr", bufs=3) as pbr,
                tc.tile_pool(name="psS", bufs=3, space="PSUM") as psS,
                tc.tile_pool(name="psR", bufs=2, space="PSUM") as psR,
                tc.tile_pool(name="psA2", bufs=2, space="PSUM") as psA2,
                tc.tile_pool(name="psN", bufs=1, space="PSUM") as psN,
            ):
                if mask_mode == "causal":
                    pmask = [pat.tile([128, ST], F16, name=f"pm{r}", tag=f"pm{r}") for r in range(4)]
                    for r in range(4):
                        nc.sync.dma_start(pmask[r][:], pmask_d[r])
                # o-proj weights after the pmask tiles on the sync queue
                for hc in range(H):
                    nc.sync.dma_start(wo[hc][:], wo_d[hc * 128:(hc + 1) * 128, :])
                for h in range(HPG):
                    qtr_t = qTr[h // 2]
                    ro = (h % 2) * ROPE
                    for qb in range(NST):
                        qsl = slice(qb * ST, (qb + 1) * ST)
                        nkt = 4 * (qb + 1) if mask_mode == "causal" else NTT
                        ps_at = psA2.tile([128, ST], F32, name="psat", tag="psat")
                        pts = [ptp.tile([128, ST], F16, name=f"ptsum{x}", tag=f"ptsum{x}", bufs=2)
                               for x in range(2)]
                        for kt in range(nkt):
                            ps = psS.tile([128, ST], F32, name="pss", tag="pss")
                            ksl = slice(kt * 128, (kt + 1) * 128)
                            nc.tensor.matmul(ps[:], kTn[h][:, ksl], qTn[h][:, qsl],
                                             start=True, stop=False)
                            nc.tensor.matmul(ps[:], kpe_both[ro:ro + ROPE, ksl],
                                             qtr_t[ro:ro + ROPE, qsl],
                                             start=False, stop=True)
                            if mask_mode == "generic":
                                mt = ptp.tile([128, ST], F32, name="mt", tag="mt")
                                nc.sync.dma_start(mt[:], maskT_d[ksl, qsl])
                                nc.vector.tensor_add(ps[:], ps[:], mt[:])
                            pt = ptp.tile([128, ST], F16, name="pt", tag="pt")
                            nc.scalar.activation(pt[:], ps[:], AF.Exp)
                            if mask_mode == "causal" and kt >= 4 * qb:
                                nc.vector.tensor_mul(pt[:], pt[:], pmask[kt % 4][:])
                            # probability sums accumulate on the DVE, keeping
                            # the PE free for score/PV matmuls
                            if kt == 0:
                                nc.vector.tensor_copy(pts[0][:], pt[:])
                            else:
                                nc.vector.tensor_add(pts[kt % 2][:], pts[1 - kt % 2][:], pt[:])
                            nc.tensor.matmul(ps_at[:], Vn[kt][:, h * VDIM:(h + 1) * VDIM],
                                             pt[:], start=(kt == 0), stop=(kt == nkt - 1))
                        # normalize: one row-sum matmul on the accumulated
                        # probabilities, broadcast, fast-reciprocal, fused mul
                        ps_rs = psR.tile([1, ST], F32, name="psrs", tag="psrs")
                        nc.tensor.matmul(ps_rs[:], ones_col[:], pts[(nkt - 1) % 2][:],
                                         start=True, stop=True)
                        rs16 = pat.tile([1, ST], F16, name="rs16", tag="rs16")
                        nc.any.tensor_copy(rs16[:], ps_rs[:])
                        psb = psN.tile([128, ST], F32, name="psn", tag="psn")
                        nc.tensor.matmul(psb[:], ones_row[:], rs16[:], start=True, stop=True)
                        bcr = pbr.tile([128, ST], F32, name="bcr", tag="bcr")
                        nc.vector.reciprocal_approx_fast(bcr[:], psb[:])
                        nc.vector.tensor_mul(attnT[h][:, qsl], ps_at[:], bcr[:])
                    # ship this head's attnT while later heads compute, and
                    # prefetch its gathered slices for the o-proj right behind
                    nc.gpsimd.dma_start(agat_src[h][:], attnT[h][:])
                    nc.gpsimd.collective_compute(
                        "AllGather", mybir.AluOpType.bypass, replica_groups=GROUPS,
                        ins=[agat_src[h].opt()], outs=[agat_dst[h].opt()])
                    # gathered-slice prefetch on the sync queue (idle after the
                    # wo loads); on gpsimd it would block the next head's
                    # staging until this AllGather completes
                    for m in range(NST):
                        nc.sync.dma_start(
                            atg[m * 4 + h][:],
                            agat_dst[h][m, :, bass.ds(toff, ST)])

            # ------- o-proj: slice own tokens from gathered heads, full contract.
            # Two passes: heads 0-11 accumulate into SBUF partials while head 3's
            # AllGather is still in flight; heads 12-15 + write-out afterwards.
            with (
                tc.tile_pool(name="oloop", bufs=3) as pol,
                tc.tile_pool(name="opart", bufs=1) as pop,
                tc.tile_pool(name="psO", bufs=2, space="PSUM") as psO,
            ):
                opart = [pop.tile([128, ST], F16, name=f"opart{i}", tag=f"opart{i}")
                         for i in range(16)]
                # pass 1: heads whose per-head AllGather (local index 0-2) has
                # landed; runs while the local-index-3 AllGather is in flight
                pass1 = [hc for hc in range(H) if hc % 4 != 3]
                pass2 = [hc for hc in range(H) if hc % 4 == 3]
                for ncol in range(4):
                    csl = slice(ncol * ST, (ncol + 1) * ST)
                    for tl in range(4):
                        ps = psO.tile([128, ST], F32, name="pso", tag="pso")
                        for i, hc in enumerate(pass1):
                            nc.tensor.matmul(ps[:], atg[hc][:, tl * 128:(tl + 1) * 128],
                                             wo[hc][:, csl],
                                             start=(i == 0), stop=(i == len(pass1) - 1))
                        nc.vector.tensor_copy(opart[ncol * 4 + tl][:], ps[:])
                for ncol in range(4):
                    csl = slice(ncol * ST, (ncol + 1) * ST)
                    for tl in range(4):
                        ps = psO.tile([128, ST], F32, name="pso", tag="pso")
                        for i, hc in enumerate(pass2):
                            nc.tensor.matmul(ps[:], atg[hc][:, tl * 128:(tl + 1) * 128],
                                             wo[hc][:, csl],
                                             start=(i == 0), stop=(i == len(pass2) - 1))
                        ot = pol.tile([128, ST], F16, name="ot", tag="ot")
                        nc.vector.tensor_add(ot[:], ps[:], opart[ncol * 4 + tl][:])
                        nc.sync.dma_start(o_d[tl * 128:(tl + 1) * 128, csl], ot[:])
            oproj_pool.__exit__(None, None, None)

    # populate .instr bytes for extended-inst InstISA subclasses (the
    # custom-DVE reciprocal_approx_fast) — raw Bass skips this pass
    mybir.codegen_inst_isa_subclasses(nc)
    _split_multi_waits(nc)
    return nc


_CACHE = {}


def _get_program(mask_mode):
    if mask_mode not in _CACHE:
        _CACHE[mask_mode] = _build_program(mask_mode)
    return _CACHE[mask_mode]


def _host_prep(hidden_states, attention_mask, position_ids, w_qa, qa_ln_w, w_qb,
               w_kva, kva_ln_w, w_kvb, w_o):
    f16 = np.float16
    mask2d = np.asarray(attention_mask, np.float32).reshape(S, S)
    causal_ref = np.triu(np.full((S, S), -1e9, np.float32), k=1)
    if np.array_equal(mask2d, causal_ref):
        mask_mode = "causal"
    elif not mask2d.any():
        mask_mode = "none"
    else:
        mask_mode = "generic"

    # weight prep: fold RMSNorm gains into B-projections, SCALE into q side
    w_qb_eff = (np.asarray(w_qb, np.float32) * np.asarray(qa_ln_w, np.float32)[:, None]) * SCALE
    w_kvb_eff = np.asarray(w_kvb, np.float32) * np.asarray(kva_ln_w, np.float32)[:, None]
    wqb3 = w_qb_eff.reshape(QR, H, QHD)
    wkvb3 = w_kvb_eff.reshape(KVR, H, NOPE + VDIM)

    pos = np.asarray(position_ids).astype(np.int64)
    inv_freq = 1.0 / (THETA ** (np.arange(0, ROPE, 2, dtype=np.float32) / ROPE))
    t = np.arange(S, dtype=np.float32)
    freqs = np.outer(t, inv_freq)
    emb = np.concatenate([freqs, freqs], axis=-1)   # [S, ROPE]
    cosT = np.cos(emb)[pos].T.astype(f16)           # [ROPE, S]
    sinT = np.sin(emb)[pos].T.astype(f16)
    cos2 = np.ascontiguousarray(np.concatenate([cosT, cosT], axis=0))  # [128, S]
    sin2 = np.ascontiguousarray(np.concatenate([sinT, sinT], axis=0))

    # causal keep-mask patterns for the transposed diagonal tiles:
    # keep iff 128*r + ki <= qj  (r = kt % 4)
    ki = np.arange(128)[:, None]
    qj = np.arange(ST)[None, :]
    pmaskT = np.stack([(128 * r + ki <= qj) for r in range(4)]).astype(f16)

    wqa16 = np.asarray(w_qa, np.float32).astype(f16)
    wkva16 = np.asarray(w_kva, np.float32).astype(f16)

    hiddenT = [np.ascontiguousarray(np.asarray(hidden_states[b], np.float32).T).astype(f16)
               for b in range(B)]
    wo_full = np.asarray(w_o, np.float32).astype(f16)

    in_maps = []
    for c in range(8):
        b, g = divmod(c, 4)
        hs = range(g * HPG, (g + 1) * HPG)
        m = {
            "hiddenT": np.ascontiguousarray(hiddenT[b][:, g * ST:(g + 1) * ST]),
            "wqa": wqa16,
            "wkva": wkva16,
            "wqbn": np.ascontiguousarray(
                np.concatenate([wqb3[:, h, :NOPE] for h in hs], axis=1)).astype(f16),
            "wqbr": np.ascontiguousarray(
                np.concatenate([wqb3[:, h, NOPE:] for h in hs], axis=1)).astype(f16),
            "wkvbk": np.ascontiguousarray(
                np.concatenate([wkvb3[:, h, :NOPE] for h in hs], axis=1)).astype(f16),
            "wkvbv": np.ascontiguousarray(
                np.concatenate([wkvb3[:, h, NOPE:] for h in hs], axis=1)).astype(f16),
            "wo": wo_full,
            "cos2": cos2,
            "sin2": sin2,
        }
        if mask_mode == "causal":
            m["pmaskT"] = pmaskT
        if mask_mode == "generic":
            m["maskT"] = np.ascontiguousarray(mask2d.T)
        in_maps.append(m)
    return mask_mode, in_maps


def kernel(hidden_states, attention_mask, position_ids, w_qa, qa_ln_w, w_qb,
           w_kva, kva_ln_w, w_kvb, w_o, _want_trace=False, _trace_kwargs=None):
    mask_mode, in_maps = _host_prep(
        hidden_states, attention_mask, position_ids, w_qa, qa_ln_w, w_qb,
        w_kva, kva_ln_w, w_kvb, w_o)
    nc = _get_program(mask_mode)
    kwargs = {}
    if _want_trace:
        kwargs.update(trace=True, **(_trace_kwargs or {}))
    res = run_bass_kernel_spmd(nc, in_maps, list(range(8)), **kwargs)
    out = np.empty((B, S, D), np.float32)
    for c in range(8):
        b, g = divmod(c, 4)
        out[b, g * ST:(g + 1) * ST, :] = res.results[c]["o_part"].astype(np.float32)
    if _want_trace:
        kernel._last_result = res
    return out
